# revision 10
# baseline (speedup 1.0000x reference)
"""Trainium2 Bass kernel for nn_CompressiveMemory_57750130262084.

The reference computes (B=8, S=4096, DK=DV=1024):
    sigma  = elu(query) + 1                                  [B,S,DK]
    memory = einsum('bkd,bsv->bkv', swap(sigma), value)      [B,DK,DV]
    z_norm = sum_s sigma                                     [B,DK]
    out    = einsum('bsd,bkv->bsv', sigma, memory)
           / einsum('bsd,bk->bs',  sigma, z_norm)[..., None]

Every einsum uses disjoint summed subscripts, so each factorises into
outer products of independent reductions; everything cancels except
    out[b,s,v] = sum_s value[b,s,v]     (exactly; query cancels)

So the kernel is a column-sum of `value` over S, broadcast over S.
Sharding: data-parallel over batch, one NeuronCore per batch element.
Per-core work: read 16 MB, reduce 4096 rows -> 1 row, write 16 MB.

Measured facts driving this schedule (NTFF traces on this pod):
  - Only full 128-partition DMAs hit the fast descriptor path
    (~146 ns read / ~162 ns write per 4 KB packet, all 16 SDMA
    engines ~100%% busy).  Partial-partition DMAs ([120,X], [92,X],
    2D-partition APs) degrade EVERY packet in the stream to ~270 ns -
    measured, so no partition-level rebalancing is possible.
  - SDMA engine 15 is ~14%% slower than the rest and straggles each
    phase by ~5-6 us.  Structural; absorbed into the budget.
  - f32 matmul = 2 HW passes per instruction: a [128,1024] slot costs
    ~1.7 us on the PE vs ~1.23 us on the DVE; slot line rate is
    ~1.25 us.  Neither engine alone keeps pace with slack, so slots
    are split ~2:1 DVE:PE (the mix the baseline sustained 405 GB/s
    with), interleaved so neither engine ever backlogs.
  - DMA completion semaphores fire ~2.5 us after the last byte, and
    consumers wait on whole-DMA sems, so every read DMA is one
    512 KB slot: consumers trail the stream by one slot + receipt
    instead of a 4 MB batch.

Schedule per core:
  - 32 x 512 KB read DMAs on the sync HWDGE queue.
  - DVE chains 21 slots into acc; PE matmul-reduces 11 slots into
    PSUM (ones^T accumulating matmuls).  The last slot is DVE's, the
    acc fold into PSUM is the only work after it: tail after the last
    input semaphore is ~1.2 us (add) + ~1.7 (fold) + ~0.7 (copy).
  - PSUM -> SBUF copy in halves (DVE + ACT in parallel); the ACT
    table is preloaded by a dummy scalar.copy at t=0 (the lazy
    ACT_TABLE_LOAD costs 1.3 us on the critical path otherwise).
  - 4 x 4 MB broadcast write DMAs on the scalar HWDGE queue (separate
    logical queue from the reads), step-0 source AP fanning the
    [128,1024] colsum tile to all 4096 rows.
"""

import numpy as np

B, S, D = 8, 4096, 1024
P = 128                 # SBUF partitions
H = 512                 # PSUM bank width in f32 (matmul N limit)
N_SLOT = S // P         # 32 x [128,1024] slots
PE_SLOTS = tuple(range(3, 28, 3)) + (30,)   # 10 slots on the PE; rest DVE
# (slot 30 keeps the PE pipeline warm right before the fold, so the
#  fold's 4 instructions run at ~0.43us cadence instead of cold ~1.1us)
OUT_REP = 8             # row-slots per output DMA -> 4 MB writes
N_OUT = N_SLOT // OUT_REP

_CACHE: dict = {}


def _build_program():
    import concourse.mybir as mybir
    import concourse.tile as tile
    from concourse import bacc

    f32 = mybir.dt.float32
    nc = bacc.Bacc("TRN2", target_bir_lowering=False, debug=False, num_devices=B, enable_asserts=False)
    v = nc.declare_dram_parameter("value", [S, D], f32, isOutput=False)
    o = nc.declare_dram_parameter("out", [S, D], f32, isOutput=True)
    vf, of = v[:], o[:]

    with tile.TileContext(nc) as tcx:
        with (
            tcx.tile_pool(name="in", bufs=1) as in_pool,
            tcx.tile_pool(name="acc", bufs=1) as acc_pool,
            tcx.tile_pool(name="ones", bufs=1) as ones_pool,
            tcx.tile_pool(name="bcast", bufs=1) as bcast_pool,
            tcx.tile_pool(name="warm", bufs=1) as warm_pool,
            tcx.tile_pool(name="psum", bufs=1, space="PSUM") as psum_pool,
        ):
            # Preload the ACT table so the tail-time scalar.copy is cheap.
            warm = warm_pool.tile([P, 2], f32)
            nc.vector.memset(warm[:], 0.0)
            nc.scalar.copy(warm[:, 0:1], warm[:, 1:2])

            ones = ones_pool.tile([P, P], f32)
            nc.vector.memset(ones[:], 1.0)

            # ---- input: one 512 KB full-width DMA per slot (sync queue).
            tiles = []
            for k in range(N_SLOT):
                t = in_pool.tile([P, D], f32, tag=f"s{k}")
                nc.sync.dma_start(t[:], vf[k * P : (k + 1) * P])
                tiles.append(t)

            ps = psum_pool.tile([P, D], f32)

            def mm(moving, start, stop):
                for h in range(2):
                    nc.tensor.matmul(
                        ps[:, h * H : (h + 1) * H],
                        ones[:],
                        moving[:, h * H : (h + 1) * H],
                        start=start,
                        stop=stop,
                    )

            # PE slots (interleaved so the PE never backlogs; ~1.7us per
            # slot at a ~3.7us assigned-arrival cadence).
            for j, k in enumerate(PE_SLOTS):
                mm(tiles[k][:], start=(j == 0), stop=False)

            # DVE slots, chained into acc; the final slot is DVE's.
            dve = [k for k in range(N_SLOT) if k not in PE_SLOTS]
            acc = acc_pool.tile([P, D], f32)
            nc.vector.tensor_copy(acc[:], tiles[dve[0]][:])
            for k in dve[1:]:
                nc.vector.tensor_add(acc[:], acc[:], tiles[k][:])

            # Fold acc into PSUM: the only post-stream PE work.
            mm(acc, start=False, stop=True)

            # PSUM -> SBUF in parallel halves.  ACT takes bank A (its
            # stop-matmul retires ~0.4us before bank B's) and DVE takes
            # bank B; emitted in that order so they truly overlap.
            bc = bcast_pool.tile([P, D], f32)
            nc.scalar.copy(bc[:, 0:H], ps[:, 0:H])
            nc.vector.tensor_copy(bc[:, H:D], ps[:, H:D])

            # ---- output: broadcast writes, issued from the sync engine
            # (idle and ring-empty by now; the scalar engine is still
            # finishing its copy when the first write must be issued).
            o_re = of.rearrange("(i n p) m -> i p n m", i=N_OUT, n=OUT_REP, p=P)
            src = bc[:].unsqueeze(1).to_broadcast((P, OUT_REP, D))
            for i in range(N_OUT):
                nc.sync.dma_start(o_re[i], src)

    nc.compile()
    return nc


def _get_program():
    if "nc" not in _CACHE:
        _CACHE["nc"] = _build_program()
    return _CACHE["nc"]


def kernel(query: np.ndarray, value: np.ndarray) -> np.ndarray:
    from concourse.bass_utils import run_bass_kernel_spmd

    del query  # output is exactly independent of query (see module docstring)
    value = np.ascontiguousarray(value, dtype=np.float32)
    assert value.shape == (B, S, D)

    nc = _get_program()
    in_maps = [{"value": value[b]} for b in range(B)]
    try:
        res = run_bass_kernel_spmd(nc, in_maps, list(range(B)))
    except Exception:
        # The tunneled runtime occasionally surfaces a transient
        # NRT_EXEC_UNIT_UNRECOVERABLE on the first dispatch; retry once.
        import time

        time.sleep(2.0)
        res = run_bass_kernel_spmd(nc, in_maps, list(range(B)))
    return np.stack([res.results[b]["out"] for b in range(B)], axis=0)
